# revision 1
# baseline (speedup 1.0000x reference)
"""ChannelAttention (B,D,H,W,C = 4,8,32,32,512; 8 heads, head_dim 64) on 8
Trainium2 NeuronCores, Bass/Tile SPMD. Fully data-parallel: zero cross-core
communication.

Sharding: the 32768 tokens (B * D*H*W) are split 8 ways -> 4096 output tokens
per core; cores (2j, 2j+1) handle the two halves of batch j. Channel
attention needs the per-head 64x64 k^T v Gram matrix over ALL of a batch's
tokens, so each core redundantly computes k|v for its whole batch (8192
tokens; its own half ordered first in its xT input). This duplicated k|v work
(~55us of PE) is cheaper and far more robust than any cross-core reduction
(a ncfw AllReduce costs ~70us fixed + a 67us start barrier).

Schedule per core:
  pass A   : stream xT chunks (16 = both halves), k|v = x @ Wkv^T (k scale
             folded in on host), accumulate per-head-pair k^T v into PSUM
             (head-pair x head-quad blocking so fp32r matmuls hit the N>=256
             full-rate mode). HAM warm-up keeper matmuls run during the
             initial DMA fill.
  softmax  : pack 8 64x64 blocks, rowwise softmax over e on [128, 4, 64]
             (DVE reduce/reciprocal, ACT exp) -- overlapped with
  pass B   : qT = Wq @ x^T for the core's own 4096 tokens (re-streams x).
  phase 2  : PE-transpose probs into block-diagonal pair lhsT, out = attnT @
             qT, proj y = out @ Wproj^T (+bias via DVE broadcast add),
             software-pipelined one chunk ahead.

Numerics: all matmuls in float32r (fp32 storage, reduced-precision PE
multiply, ~13-bit effective mantissa) with fp32 PSUM accumulation; softmax in
fp32. End-to-end L2 relative error vs the fp32 reference: ~1.0e-3.
"""

import os
import numpy as np
from contextlib import ExitStack

import concourse.bass as bass
import concourse.mybir as mybir
import concourse.tile as tile
from concourse import bacc
from concourse.bass_utils import run_bass_kernel_spmd
from concourse.masks import make_identity

B, D, H, W, C = 4, 8, 32, 32, 512
NUM_HEADS = 8
HEAD_DIM = C // NUM_HEADS
SCALE = HEAD_DIM ** -0.5
N_TOK = B * D * H * W
N_CORES = 8
N_LOC = N_TOK // N_CORES
CHUNK = 512
N_CHUNKS = N_LOC // CHUNK
TT = 128
T_PER_CHUNK = CHUNK // TT
N_CI = C // 128
N_PAIRS = NUM_HEADS // 2

f32 = mybir.dt.float32
f32r = mybir.dt.float32r

N_KEEP_START = 24
N_KEEP_MID = 16

_NC_CACHE = None


def build_nc():
    nc = bacc.Bacc(num_devices=N_CORES)

    xT = nc.declare_dram_parameter("xT", [C, 2 * N_LOC], f32r, isOutput=False)
    wq = nc.declare_dram_parameter("wq", [C, C], f32r, isOutput=False)
    wkv = nc.declare_dram_parameter("wkv", [C, 2 * C], f32r, isOutput=False)
    wp = nc.declare_dram_parameter("wp", [C, C], f32r, isOutput=False)
    bp = nc.declare_dram_parameter("bp", [1, C], f32r, isOutput=False)
    y = nc.declare_dram_parameter("y", [N_LOC, C], f32, isOutput=True)

    xT_v = xT.rearrange("(a p) n -> p a n", p=128)
    wq_v = wq.rearrange("(a p) f -> p a f", p=128)
    wkv_v = wkv.rearrange("(a p) f -> p a f", p=128)
    wp_v = wp.rearrange("(a p) f -> p a f", p=128)

    with tile.TileContext(nc) as tc, ExitStack() as ctx:
        const = ctx.enter_context(tc.tile_pool(name="const", bufs=1))
        persist = ctx.enter_context(tc.tile_pool(name="persist", bufs=1))
        sb = ctx.enter_context(tc.tile_pool(name="sb", bufs=2))
        kvp = ctx.enter_context(tc.tile_pool(name="kvp", bufs=4))

        wkv_sb = const.tile([128, N_CI, 2 * C], f32r)
        nc.sync.dma_start(wkv_sb[:], wkv_v[:])
        ones_f32 = const.tile([1, 128], f32)
        nc.vector.memset(ones_f32[:], 1.0)
        ones_sb = const.tile([1, 128], f32r)
        nc.vector.tensor_copy(ones_sb[:], ones_f32[:])
        zrow_f32 = const.tile([1, 512], f32)
        nc.vector.memset(zrow_f32[:], 0.0)
        zrow_sb = const.tile([1, 512], f32r)
        nc.vector.tensor_copy(zrow_sb[:], zrow_f32[:])
        ident = const.tile([128, 128], f32)
        make_identity(nc, ident[:])

        qT_all = persist.tile([128, N_PAIRS, N_CHUNKS, CHUNK], f32r)
        red_sb = persist.tile([128, N_PAIRS, 64], f32)

        # ---------------- pass A: k|v + attn partial accumulation ----------
        with (
            tc.tile_pool(name="ps_kv", bufs=2, space="PSUM") as ps_kv,
            tc.tile_pool(name="ps_at", bufs=1, space="PSUM") as ps_at,
            tc.tile_pool(name="ps_q", bufs=2, space="PSUM") as ps_q,
        ):
            attn_ps = ps_at.tile([128, N_PAIRS, 256], f32)
            # bank-wide has_written seed + HAM warm-up
            for i in range(max(2, N_KEEP_START)):
                bank = i % 2
                nc.tensor.matmul(
                    attn_ps[:, 2 * bank:2 * bank + 2, :].rearrange("p a e -> p (a e)"),
                    ones_sb[:], zrow_sb[:],
                    start=(i < 2), stop=False, skip_group_check=True,
                )

            for c in range(2 * N_CHUNKS):
                xt = sb.tile([128, N_CI, CHUNK], f32r, tag="xt")
                nc.sync.dma_start(xt[:], xT_v[:, :, c * CHUNK:(c + 1) * CHUNK])

                kv_tiles = []
                for s in range(T_PER_CHUNK):
                    kv_ps = ps_kv.tile([128, 2 * C], f32, tag="kv")
                    for h in range(2):
                        for k in range(N_CI):
                            nc.tensor.matmul(
                                kv_ps[:, h * C:(h + 1) * C],
                                xt[:, k, s * TT:(s + 1) * TT],
                                wkv_sb[:, k, h * C:(h + 1) * C],
                                start=(k == 0), stop=(k == N_CI - 1),
                            )
                    kv_sb = kvp.tile([128, 2 * C], f32r, tag="kvsb")
                    nc.vector.tensor_copy(kv_sb[:], kv_ps[:])
                    kv_tiles.append(kv_sb)

                for s in range(T_PER_CHUNK):
                    kv_sb = kv_tiles[s]
                    for p in range(N_PAIRS):
                        q4 = p // 2
                        nc.tensor.matmul(
                            attn_ps[:, p, :],
                            kv_sb[:, p * 128:(p + 1) * 128],
                            kv_sb[:, C + q4 * 256:C + (q4 + 1) * 256],
                            start=False,
                            stop=(c == 2 * N_CHUNKS - 1 and s == T_PER_CHUNK - 1),
                            skip_group_check=True,
                        )

            # pack 8 useful 64x64 blocks -> red_sb[d + 64*(h%2), h//2, :]
            for h in range(NUM_HEADS):
                p = h // 2
                row0 = (h % 2) * 64
                col0 = (p % 2) * 128 + row0
                nc.vector.tensor_copy(
                    red_sb[row0:row0 + 64, h // 2, :],
                    attn_ps[row0:row0 + 64, p, col0:col0 + 64],
                )

            # weights for pass B / phase 2 (loaded once pass A's DMAs drain)
            wq_sb = const.tile([128, N_CI, C], f32r)
            nc.sync.dma_start(wq_sb[:], wq_v[:])
            wp_sb = const.tile([128, N_CI, C], f32r)
            nc.sync.dma_start(wp_sb[:], wp_v[:])
            bp_f32 = const.tile([128, C], f32)
            bp_bcast = bass.AP(
                tensor=bp[:].bitcast(f32).tensor,
                offset=0,
                ap=[[0, 128], [1, C]],
            )
            nc.sync.dma_start(bp_f32[:], bp_bcast)

            # ---- softmax over e on [128, pair, 64] (overlaps pass B) ----
            nmax = sb.tile([128, N_PAIRS, 1], f32, tag="nmax")
            nc.vector.reduce_max(nmax[:], red_sb[:], axis=mybir.AxisListType.X, negate=True)
            shifted = sb.tile([128, N_PAIRS, 64], f32, tag="shifted")
            nc.vector.tensor_add(shifted[:], red_sb[:], nmax.broadcast_to([128, N_PAIRS, 64]))
            expd = sb.tile([128, N_PAIRS, 64], f32, tag="expd")
            nc.scalar.activation(expd[:], shifted[:], mybir.ActivationFunctionType.Exp)
            ssum = sb.tile([128, N_PAIRS, 1], f32, tag="ssum")
            nc.vector.reduce_sum(ssum[:], expd[:], axis=mybir.AxisListType.X)
            rsum = sb.tile([128, N_PAIRS, 1], f32, tag="rsum")
            nc.vector.reciprocal(rsum[:], ssum[:])
            probs = sb.tile([128, N_PAIRS, 64], f32, tag="probs")
            nc.vector.tensor_mul(probs[:], expd[:], rsum.broadcast_to([128, N_PAIRS, 64]))
            probs2 = sb.tile([64, NUM_HEADS, 64], f32, tag="probs2")
            nc.vector.tensor_copy(probs2[:, 0::2, :], probs[0:64, :, :])
            nc.vector.tensor_copy(probs2[:, 1::2, :], probs[64:128, :, :])
            zro = sb.tile([128, N_PAIRS, 128], f32, tag="zro")
            nc.vector.memset(zro[:], 0.0)
            atnT = persist.tile([128, N_PAIRS, 128], f32r)
            nc.vector.tensor_copy(atnT[:], zro[:])

            # ------------- pass B: qT (overlaps the exchange) --------------
            for c in range(N_CHUNKS):
                xt = sb.tile([128, N_CI, CHUNK], f32r, tag="xtb")
                nc.sync.dma_start(xt[:], xT_v[:, :, c * CHUNK:(c + 1) * CHUNK])
                for p in range(N_PAIRS):
                    q_ps = ps_q.tile([128, CHUNK], f32, tag="q")
                    for k in range(N_CI):
                        nc.tensor.matmul(
                            q_ps[:],
                            wq_sb[:, k, p * 128:(p + 1) * 128],
                            xt[:, k, :],
                            start=(k == 0), stop=(k == N_CI - 1),
                        )
                    nc.scalar.copy(qT_all[:, p, c, :], q_ps[:])

        with (
            tc.tile_pool(name="ps_tr", bufs=1, space="PSUM") as ps_tr,
            tc.tile_pool(name="ps_keep", bufs=1, space="PSUM") as ps_keep,
            tc.tile_pool(name="ps_o", bufs=3, space="PSUM") as ps_o,
            tc.tile_pool(name="ps_y", bufs=3, space="PSUM") as ps_y,
        ):
            # HAM keepers in case the exchange outlasts pass B
            keep_ps = ps_keep.tile([128, C], f32)
            for i in range(N_KEEP_MID):
                nc.tensor.matmul(
                    keep_ps[:], ones_sb[:], zrow_sb[:],
                    start=(i == 0), stop=False, skip_group_check=True,
                )

            # ---- transpose probs -> block-diag pair lhsT (f32r) ----
            tr_ps = ps_tr.tile([64, NUM_HEADS, 64], f32)
            for h in range(NUM_HEADS):
                nc.tensor.transpose(tr_ps[:, h, :], probs2[:, h, :], ident[0:64, 0:64])
            for h in range(NUM_HEADS):
                p = h // 2
                off = (h % 2) * 64
                nc.vector.tensor_copy(
                    atnT[off:off + 64, p, off:off + 64], tr_ps[:, h, :]
                )

            # ---------------- phase 2: out + proj --------------------------
            def emit_out(c):
                outT_sb = sb.tile([128, N_CI, CHUNK], f32r, tag="outT", bufs=3, name=f"outT_{c}")
                for p in range(N_PAIRS):
                    o_ps = ps_o.tile([128, CHUNK], f32, tag="o", name=f"o_{c}_{p}")
                    nc.tensor.matmul(
                        o_ps[:], atnT[:, p, :], qT_all[:, p, c, :],
                        start=True, stop=True,
                    )
                    nc.scalar.copy(outT_sb[:, p, :], o_ps[:])
                return outT_sb

            outT_tiles = {0: emit_out(0), 1: emit_out(1)}
            for c in range(N_CHUNKS):
                if c + 2 < N_CHUNKS:
                    outT_tiles[c + 2] = emit_out(c + 2)
                outT_sb = outT_tiles.pop(c)
                for s in range(T_PER_CHUNK):
                    y_ps = ps_y.tile([128, C], f32, tag="y")
                    for k in range(N_CI):
                        nc.tensor.matmul(
                            y_ps[:],
                            outT_sb[:, k, s * TT:(s + 1) * TT],
                            wp_sb[:, k, :],
                            start=(k == 0), stop=(k == N_CI - 1),
                        )
                    y_sb = sb.tile([128, C], f32, tag="ysb", bufs=4)
                    nc.vector.tensor_add(y_sb[:], y_ps[:], bp_f32[:])
                    t0 = c * CHUNK + s * TT
                    nc.sync.dma_start(y[t0:t0 + TT, :], y_sb[:])

    nc.compile()
    return nc


def _get_nc():
    global _NC_CACHE
    if _NC_CACHE is None:
        _NC_CACHE = build_nc()
    return _NC_CACHE


def prep_inputs(x, Wqkv, Wproj, bproj):
    x = np.ascontiguousarray(np.asarray(x, dtype=np.float32))
    Wqkv = np.asarray(Wqkv, dtype=np.float32)
    Wproj = np.asarray(Wproj, dtype=np.float32)
    bproj = np.asarray(bproj, dtype=np.float32)

    xf = x.reshape(B, D * H * W, C)
    wq = np.ascontiguousarray(Wqkv[0:C].T)
    wk = Wqkv[C:2 * C] * np.float32(SCALE)
    wv = Wqkv[2 * C:3 * C]
    wkv = np.ascontiguousarray(np.concatenate([wk, wv], axis=0).T)
    wp = np.ascontiguousarray(Wproj.T)
    bp = np.ascontiguousarray(bproj.reshape(1, C))

    in_maps = []
    for i in range(N_CORES):
        b = i // 2
        t0 = (i % 2) * N_LOC
        own = xf[b, t0:t0 + N_LOC, :]
        pair = xf[b, N_LOC - t0:2 * N_LOC - t0, :]
        xTl = np.ascontiguousarray(np.concatenate([own, pair], axis=0).T)
        in_maps.append({"xT": xTl, "wq": wq, "wkv": wkv, "wp": wp, "bp": bp})
    return in_maps


def gather_output(results):
    parts = [np.asarray(results[i]["y"]) for i in range(N_CORES)]
    return np.concatenate(parts, axis=0).reshape(B, D, H, W, C)


def kernel(x, Wqkv, Wproj, bproj, _trace=False, _tmpdir=None):
    nc = _get_nc()
    in_maps = prep_inputs(x, Wqkv, Wproj, bproj)
    res = run_bass_kernel_spmd(
        nc, in_maps, list(range(N_CORES)), trace=_trace, tmpdir=_tmpdir
    )
    out = gather_output(res.results)
    if _trace:
        kernel.last_exec_time_ns = res.exec_time_ns
        kernel.last_results = res
    return out



# revision 3
# speedup vs baseline: 1.6471x; 1.6471x over previous
"""ChannelAttention (B,D,H,W,C = 4,8,32,32,512; 8 heads, head_dim 64) on 8
Trainium2 NeuronCores, Bass/Tile SPMD. Fully data-parallel: zero cross-core
communication; cores (2j, 2j+1) handle the two halves of batch j.

Algebraic collapse: channel attention mixes only the channel dim per token,
so the whole module is one batch-dependent linear map

    y = X @ M^T + b,   M = sum_h Wp_h A_h Wq_h,
    A_h = softmax(Wk_h G Wv_h^T),   G = X^T X   (512x512, all heads share it)

with X = the batch's 8192xC tokens and SCALE folded into Wk on host. Per-core
PE work drops from ~15.6 GFLOP (qkv projection over 8192 tokens + attention +
proj) to ~7 GFLOP:

  phase 1: G = X^T X   — 256 fp32r matmuls 128x128x512 over streamed
           token-major x; xT (channel-major own half) and the four weight
           mats prefetched in the DMA gaps; HAM keepers cover the fill.
  phase 2: V = G Wv^T (16 mm; G's symmetry supplies the transposed lhsT),
           logits = Wk V with head-pair x head-quad blocking so fp32r hits
           the N>=256 full-rate mode (16 mm), rowwise softmax on [128,4,64]
           (DVE reduce/reciprocal, ACT exp), PE-transpose of A into
           block-diag pair lhsT, B'_p = A_p Wq_p (4 mm), M^T accumulated
           from (B'_p, Wp_p) pairs (16 mm). PSUM->SBUF copies alternate
           DVE/ACT engines; keepers bridge the PE idle gaps.
  phase 3: y = X M^T + b — 128 matmuls 128x128x512 on the SBUF-resident xT,
           bias via DVE broadcast add, streamed DMA out.

Numerics: all matmuls float32r (fp32 storage, ~13-bit effective mantissa)
with fp32 PSUM accumulation; softmax fp32. The logits (std ~87) match the
baseline two-pass scheme's precision since G accumulates the same 8192-token
sums. End-to-end L2 relative error vs the fp32 reference: ~1e-3.
"""

import numpy as np
from contextlib import ExitStack

import concourse.bass as bass
import concourse.mybir as mybir
import concourse.tile as tile
from concourse import bacc
from concourse.bass_utils import run_bass_kernel_spmd
from concourse.masks import make_identity

B, D, H, W, C = 4, 8, 32, 32, 512
NUM_HEADS = 8
HEAD_DIM = C // NUM_HEADS
SCALE = HEAD_DIM ** -0.5
N_TOK = B * D * H * W          # 32768
N_CORES = 8
N_B = N_TOK // B               # 8192 tokens per batch
N_LOC = N_B // 2               # 4096 own tokens per core
N_CI = C // 128                # 4 channel blocks
N_PAIRS = NUM_HEADS // 2       # 4
CHUNK = 512                    # tokens per DMA chunk
G_CHUNKS = N_B // CHUNK        # 16
Y_CHUNKS = N_LOC // CHUNK      # 8
TT = 128
T_PER_CHUNK = CHUNK // TT      # 4

f32 = mybir.dt.float32
f32r = mybir.dt.float32r

N_KEEP_START = 10
N_KEEP_SOFTMAX = 6

_NC_CACHE = None


def build_nc():
    nc = bacc.Bacc(num_devices=N_CORES)

    xr = nc.declare_dram_parameter("xr", [N_B, C], f32r, isOutput=False)
    xT = nc.declare_dram_parameter("xT", [C, N_LOC], f32r, isOutput=False)
    wk = nc.declare_dram_parameter("wk", [C, C], f32r, isOutput=False)
    wv = nc.declare_dram_parameter("wv", [C, C], f32r, isOutput=False)
    wq = nc.declare_dram_parameter("wq", [C, C], f32r, isOutput=False)
    wp = nc.declare_dram_parameter("wp", [C, C], f32r, isOutput=False)
    bp = nc.declare_dram_parameter("bp", [1, C], f32r, isOutput=False)
    y = nc.declare_dram_parameter("y", [N_LOC, C], f32, isOutput=True)

    xr_v = xr.rearrange("(t p) c -> p t c", p=128)    # [128, 64, 512]
    xT_v = xT.rearrange("(a p) n -> p a n", p=128)    # [128, 4, 4096]
    wk_v = wk.rearrange("(a p) f -> p a f", p=128)
    wv_v = wv.rearrange("(a p) f -> p a f", p=128)
    wq_v = wq.rearrange("(a p) f -> p a f", p=128)
    wp_v = wp.rearrange("(a p) f -> p a f", p=128)

    with tile.TileContext(nc) as tc, ExitStack() as ctx:
        const = ctx.enter_context(tc.tile_pool(name="const", bufs=1))
        persist = ctx.enter_context(tc.tile_pool(name="persist", bufs=1))
        sb = ctx.enter_context(tc.tile_pool(name="sb", bufs=2))

        ones_f32 = const.tile([1, 128], f32)
        nc.vector.memset(ones_f32[:], 1.0)
        ones_sb = const.tile([1, 128], f32r)
        nc.vector.tensor_copy(ones_sb[:], ones_f32[:])
        zrow_f32 = const.tile([1, 512], f32)
        nc.vector.memset(zrow_f32[:], 0.0)
        zrow_sb = const.tile([1, 512], f32r)
        nc.vector.tensor_copy(zrow_sb[:], zrow_f32[:])
        ident = const.tile([128, 128], f32)
        make_identity(nc, ident[:])

        xT_sb = persist.tile([128, N_CI, N_LOC], f32r)      # 8MB, phase-3 lhsT
        G_sb = persist.tile([128, N_CI, C], f32r)
        V_sb = persist.tile([128, N_CI, C], f32r)
        Bp_sb = persist.tile([128, N_PAIRS, C], f32r)
        MT_sb = persist.tile([128, N_CI, C], f32r)
        red_sb = persist.tile([128, N_PAIRS, 64], f32)
        atnT = persist.tile([128, N_PAIRS, 128], f32r)

        wk_sb = const.tile([128, N_CI, C], f32r)
        wv_sb = const.tile([128, N_CI, C], f32r)
        wq_sb = const.tile([128, N_CI, C], f32r)
        wp_sb = const.tile([128, N_CI, C], f32r)
        bp_f32 = const.tile([128, C], f32)

        # zero atnT early (block-diag lhsT scaffold for the A^T transposes)
        zro = sb.tile([128, N_PAIRS, 128], f32, tag="zro")
        nc.vector.memset(zro[:], 0.0)
        nc.vector.tensor_copy(atnT[:], zro[:])

        # ---------------- phase 1: G = X^T X ------------------------------
        with (
            tc.tile_pool(name="ps_g", bufs=1, space="PSUM") as ps_g,
            tc.tile_pool(name="ps_keep", bufs=1, space="PSUM") as ps_keep,
            tc.tile_pool(name="ps_v", bufs=2, space="PSUM") as ps_v,
        ):
            G_ps = ps_g.tile([128, N_CI, C], f32)
            keep_ps = ps_keep.tile([128, C], f32)
            for i in range(N_KEEP_START):
                nc.tensor.matmul(
                    keep_ps[:], ones_sb[:], zrow_sb[:],
                    start=(i == 0), stop=False, skip_group_check=True,
                )

            xtiles = {}

            def issue_chunk(t):
                xt = sb.tile([128, T_PER_CHUNK, C], f32r, tag="xr")
                nc.sync.dma_start(
                    xt[:], xr_v[:, t * T_PER_CHUNK:(t + 1) * T_PER_CHUNK, :]
                )
                xtiles[t] = xt

            issue_chunk(0)
            issue_chunk(1)
            for t in range(G_CHUNKS):
                xt = xtiles.pop(t)
                for s in range(T_PER_CHUNK):
                    for ci in range(N_CI):
                        nc.tensor.matmul(
                            G_ps[:, ci, :],
                            xt[:, s, ci * 128:(ci + 1) * 128],
                            xt[:, s, :],
                            start=(t == 0 and s == 0),
                            stop=(t == G_CHUNKS - 1 and s == T_PER_CHUNK - 1),
                            skip_group_check=True,
                        )
                if t + 2 < G_CHUNKS:
                    issue_chunk(t + 2)
                # aux DMAs in the bandwidth slack: weights early, xT spread out
                if t == 0:
                    nc.sync.dma_start(wk_sb[:], wk_v[:])
                elif t == 2:
                    nc.sync.dma_start(wv_sb[:], wv_v[:])
                elif t == 4:
                    nc.sync.dma_start(wq_sb[:], wq_v[:])
                elif t == 6:
                    nc.sync.dma_start(wp_sb[:], wp_v[:])
                elif t == 8:
                    bp_bcast = bass.AP(
                        tensor=bp[:].bitcast(f32).tensor,
                        offset=0,
                        ap=[[0, 128], [1, C]],
                    )
                    nc.sync.dma_start(bp_f32[:], bp_bcast)
                elif t % 2 == 1:
                    u = t // 2
                    nc.sync.dma_start(
                        xT_sb[:, :, u * CHUNK:(u + 1) * CHUNK],
                        xT_v[:, :, u * CHUNK:(u + 1) * CHUNK],
                    )

            # 1 keeper bridges the G->SBUF copy latency
            nc.tensor.matmul(
                keep_ps[:], ones_sb[:], zrow_sb[:],
                start=False, stop=False, skip_group_check=True,
            )

            # G -> SBUF (f32r), alternating engines
            for ccb in range(N_CI):
                if ccb % 2 == 0:
                    nc.vector.tensor_copy(G_sb[:, ccb, :], G_ps[:, ccb, :])
                else:
                    nc.scalar.copy(G_sb[:, ccb, :], G_ps[:, ccb, :])

            # ---- V = G Wv^T (ci-major so V blocks finish early) ----
            for ci in range(N_CI):
                v_ps = ps_v.tile([128, C], f32, tag="v")
                for ccb in range(N_CI):
                    nc.tensor.matmul(
                        v_ps[:],
                        G_sb[:, ccb, ci * 128:(ci + 1) * 128],
                        wv_sb[:, ccb, :],
                        start=(ccb == 0), stop=(ccb == N_CI - 1),
                    )
                if ci % 2 == 0:
                    nc.vector.tensor_copy(V_sb[:, ci, :], v_ps[:])
                else:
                    nc.scalar.copy(V_sb[:, ci, :], v_ps[:])

        # ------------- phase 2: logits, softmax, B', M^T ------------------
        with tc.tile_pool(name="ps_keep2", bufs=1, space="PSUM") as ps_keep2:
            keep2 = ps_keep2.tile([128, C], f32)
            with (
                tc.tile_pool(name="ps_l", bufs=1, space="PSUM") as ps_l,
                tc.tile_pool(name="ps_tr", bufs=1, space="PSUM") as ps_tr,
            ):
                # logits, pair x quad blocked: L[d,e] per head in 64x64
                # diag blocks. One full bank per pair: start=True clears
                # has_written for the WHOLE bank, so pair regions must not
                # share banks.
                L_ps = ps_l.tile([128, N_PAIRS, 512], f32)
                for ccb in range(N_CI):
                    for p in range(N_PAIRS):
                        q = p // 2
                        nc.tensor.matmul(
                            L_ps[:, p, 0:256],
                            wk_sb[:, ccb, p * 128:(p + 1) * 128],
                            V_sb[:, ccb, q * 256:(q + 1) * 256],
                            start=(ccb == 0), stop=(ccb == N_CI - 1),
                            skip_group_check=True,
                        )

                # keepers run while DVE/ACT extract + softmax
                for i in range(N_KEEP_SOFTMAX):
                    nc.tensor.matmul(
                        keep2[:], ones_sb[:], zrow_sb[:],
                        start=(i == 0), stop=False, skip_group_check=True,
                    )

                # pack 8 useful 64x64 blocks -> red_sb[d + 64*(h%2), h//2, :]
                for h in range(NUM_HEADS):
                    p = h // 2
                    row0 = (h % 2) * 64
                    col0 = (p % 2) * 128 + row0
                    nc.vector.tensor_copy(
                        red_sb[row0:row0 + 64, p, :],
                        L_ps[row0:row0 + 64, p, col0:col0 + 64],
                    )

                # rowwise softmax over e on [128, pair, 64]
                nmax = sb.tile([128, N_PAIRS, 1], f32, tag="nmax")
                nc.vector.reduce_max(nmax[:], red_sb[:], axis=mybir.AxisListType.X, negate=True)
                shifted = sb.tile([128, N_PAIRS, 64], f32, tag="shifted")
                nc.vector.tensor_add(shifted[:], red_sb[:], nmax.broadcast_to([128, N_PAIRS, 64]))
                expd = sb.tile([128, N_PAIRS, 64], f32, tag="expd")
                nc.scalar.activation(expd[:], shifted[:], mybir.ActivationFunctionType.Exp)
                ssum = sb.tile([128, N_PAIRS, 1], f32, tag="ssum")
                nc.vector.reduce_sum(ssum[:], expd[:], axis=mybir.AxisListType.X)
                rsum = sb.tile([128, N_PAIRS, 1], f32, tag="rsum")
                nc.vector.reciprocal(rsum[:], ssum[:])
                probs = sb.tile([128, N_PAIRS, 64], f32, tag="probs")
                nc.vector.tensor_mul(probs[:], expd[:], rsum.broadcast_to([128, N_PAIRS, 64]))
                probs2 = sb.tile([64, NUM_HEADS, 64], f32, tag="probs2")
                nc.vector.tensor_copy(probs2[:, 0::2, :], probs[0:64, :, :])
                nc.vector.tensor_copy(probs2[:, 1::2, :], probs[64:128, :, :])

                # transpose A -> block-diag pair lhsT (f32r)
                tr_ps = ps_tr.tile([64, NUM_HEADS, 64], f32)
                for h in range(NUM_HEADS):
                    nc.tensor.transpose(tr_ps[:, h, :], probs2[:, h, :], ident[0:64, 0:64])
                for h in range(NUM_HEADS):
                    p = h // 2
                    off = (h % 2) * 64
                    nc.vector.tensor_copy(
                        atnT[off:off + 64, p, off:off + 64], tr_ps[:, h, :]
                    )

            with (
                tc.tile_pool(name="ps_b", bufs=2, space="PSUM") as ps_b,
                tc.tile_pool(name="ps_mt", bufs=1, space="PSUM") as ps_mt,
            ):
                MT_ps = ps_mt.tile([128, N_CI, C], f32)
                b_tiles = {}

                def emit_b(p):
                    bt = ps_b.tile([128, C], f32, tag="b")
                    nc.tensor.matmul(
                        bt[:], atnT[:, p, :], wq_sb[:, p, :],
                        start=True, stop=True,
                    )
                    if p % 2 == 0:
                        nc.vector.tensor_copy(Bp_sb[:, p, :], bt[:])
                    else:
                        nc.scalar.copy(Bp_sb[:, p, :], bt[:])
                    b_tiles[p] = bt

                emit_b(0)
                emit_b(1)
                for p in range(N_PAIRS):
                    if p + 2 < N_PAIRS:
                        emit_b(p + 2)
                    b_tiles.pop(p)
                    for ci in range(N_CI):
                        nc.tensor.matmul(
                            MT_ps[:, ci, :],
                            Bp_sb[:, p, ci * 128:(ci + 1) * 128],
                            wp_sb[:, p, :],
                            start=(p == 0), stop=(p == N_PAIRS - 1),
                            skip_group_check=True,
                        )

                # keepers bridge the MT->SBUF copies
                for i in range(2):
                    nc.tensor.matmul(
                        keep2[:], ones_sb[:], zrow_sb[:],
                        start=False, stop=False, skip_group_check=True,
                    )
                for ci in range(N_CI):
                    if ci % 2 == 0:
                        nc.vector.tensor_copy(MT_sb[:, ci, :], MT_ps[:, ci, :])
                    else:
                        nc.scalar.copy(MT_sb[:, ci, :], MT_ps[:, ci, :])

        # ---------------- phase 3: y = X M^T + b --------------------------
        with tc.tile_pool(name="ps_y", bufs=4, space="PSUM") as ps_y:
            for c in range(Y_CHUNKS):
                for s in range(T_PER_CHUNK):
                    t0 = c * CHUNK + s * TT
                    y_ps = ps_y.tile([128, C], f32, tag="y")
                    for ci in range(N_CI):
                        nc.tensor.matmul(
                            y_ps[:],
                            xT_sb[:, ci, t0:t0 + TT],
                            MT_sb[:, ci, :],
                            start=(ci == 0), stop=(ci == N_CI - 1),
                        )
                    y_sb = sb.tile([128, C], f32, tag="ysb", bufs=4)
                    nc.vector.tensor_add(y_sb[:], y_ps[:], bp_f32[:])
                    nc.sync.dma_start(y[t0:t0 + TT, :], y_sb[:])

    nc.compile()
    return nc


def _get_nc():
    global _NC_CACHE
    if _NC_CACHE is None:
        _NC_CACHE = build_nc()
    return _NC_CACHE


def prep_inputs(x, Wqkv, Wproj, bproj):
    x = np.ascontiguousarray(np.asarray(x, dtype=np.float32))
    Wqkv = np.asarray(Wqkv, dtype=np.float32)
    Wproj = np.asarray(Wproj, dtype=np.float32)
    bproj = np.asarray(bproj, dtype=np.float32)

    xf = x.reshape(B, N_B, C)
    wq_rm = np.ascontiguousarray(Wqkv[0:C])                             # [j, cc]
    wk_cm = np.ascontiguousarray((Wqkv[C:2 * C] * np.float32(SCALE)).T)  # [cc, j]
    wv_cm = np.ascontiguousarray(Wqkv[2 * C:3 * C].T)                    # [cc, j]
    wp_t = np.ascontiguousarray(Wproj.T)                                 # [j, c]
    bp = np.ascontiguousarray(bproj.reshape(1, C))

    in_maps = []
    for i in range(N_CORES):
        b = i // 2
        t0 = (i % 2) * N_LOC
        xT_l = np.ascontiguousarray(xf[b, t0:t0 + N_LOC].T)
        in_maps.append({
            "xr": xf[b], "xT": xT_l, "wk": wk_cm, "wv": wv_cm,
            "wq": wq_rm, "wp": wp_t, "bp": bp,
        })
    return in_maps


def gather_output(results):
    parts = [np.asarray(results[i]["y"]) for i in range(N_CORES)]
    return np.concatenate(parts, axis=0).reshape(B, D, H, W, C)


def kernel(x, Wqkv, Wproj, bproj, _trace=False, _tmpdir=None):
    nc = _get_nc()
    in_maps = prep_inputs(x, Wqkv, Wproj, bproj)
    res = run_bass_kernel_spmd(
        nc, in_maps, list(range(N_CORES)), trace=_trace, tmpdir=_tmpdir
    )
    out = gather_output(res.results)
    if _trace:
        kernel.last_exec_time_ns = res.exec_time_ns
        kernel.last_results = res
    return out
